# revision 1
# baseline (speedup 1.0000x reference)
"""Causal self-attention (k/q swapped variant) on 8 Trainium2 NeuronCores.

Problem (hardcoded shapes): B=2, N=2048, D=1024, H=16, DH=64.
  kqv = einsum('bnd,hde->bhne', x, Wkqv) + bkqv   ; split -> k, q, v
  A[b,h,n,m] = k[b,h,n]·q[b,h,m] / sqrt(DH), causal mask m<=n, softmax over m
  sa = A @ v ; concat heads ; out = sa @ Wo + bo

Sharding: tensor-parallel over heads — each core owns 2 heads (both batch
entries), computes its partial output projection sa_local @ Wo[rows], and the
host sums the 8 partials (+bo).

Per-core device kernel layout (all matmul operands bf16, fp32 PSUM accum):
  - x is pre-transposed on host to xt[b] = x[b].T ([D, N]) so the contraction
    dim d lands on SBUF partitions.
  - k/q projections produce kT/qT in [dh, n] layout with both heads stacked on
    the partition dim (h0 rows 0:64, h1 rows 64:128) -> the two heads' K=64
    score matmuls occupy disjoint PE row groups and run concurrently.
  - scores are computed transposed, S^T[m, n] = q[m]·k[n], so softmax's
    reduction dim m sits on partitions; the denominator comes for free from
    the PV matmul by augmenting v with 64 columns of ones (psum rows 64:128
    hold the replicated row-sum). exp(S^T) only computed on the causal
    region; the 128x128 diagonal triangle is zeroed with a 0/1 mask multiply.
  - v is projected directly into [n, dh] layout (stationary = xt chunk).
  - output projection: stationary = saT column block, moving = Wo rows for the
    local heads -> natural-layout partial out [n, 1024] per block.
"""

import numpy as np
import ml_dtypes

B = 2
N = 2048
D = 1024
H = 16
DH = 64
NCORES = 8
HL = H // NCORES          # heads per core = 2
DC = D // 128             # contraction chunks = 8
NB = N // 128             # 128-row blocks = 16
NJ = N // 512             # 512-col blocks = 4

BF16 = ml_dtypes.bfloat16

_CACHE = {}


def _build():
    import concourse.bass as bass
    import concourse.mybir as mybir
    import concourse.tile as tile
    from concourse import bacc
    from contextlib import ExitStack

    f32 = mybir.dt.float32
    bf16 = mybir.dt.bfloat16

    nc = bacc.Bacc("TRN2", target_bir_lowering=False, debug=False,
                   enable_asserts=False, num_devices=NCORES)

    xt_d = nc.dram_tensor("xt", [B, D, N], bf16, kind="ExternalInput")
    wk_d = nc.dram_tensor("wk2", [D, 128], bf16, kind="ExternalInput")
    wq_d = nc.dram_tensor("wq2", [D, 128], bf16, kind="ExternalInput")
    wv_d = nc.dram_tensor("wv2", [D, 128], bf16, kind="ExternalInput")
    wo_d = nc.dram_tensor("wo2", [128, D], bf16, kind="ExternalInput")
    bk_d = nc.dram_tensor("bk2", [128, 1], f32, kind="ExternalInput")
    bq_d = nc.dram_tensor("bq2", [128, 1], f32, kind="ExternalInput")
    bvt_d = nc.dram_tensor("bvt", [128, 128], f32, kind="ExternalInput")
    m01_d = nc.dram_tensor("m01", [128, 128], bf16, kind="ExternalInput")
    out_d = nc.dram_tensor("out", [B, N, D], f32, kind="ExternalOutput")

    with tile.TileContext(nc) as tc, ExitStack() as ctx:
        const = ctx.enter_context(tc.tile_pool(name="const", bufs=1))
        xt_pool = ctx.enter_context(tc.tile_pool(name="xt", bufs=2 * DC))
        kq_pool = ctx.enter_context(tc.tile_pool(name="kq", bufs=4))
        v_pool = ctx.enter_context(tc.tile_pool(name="v", bufs=2))
        sa_pool = ctx.enter_context(tc.tile_pool(name="sa", bufs=2))
        pt_pool = ctx.enter_context(tc.tile_pool(name="pt", bufs=4))
        rc_pool = ctx.enter_context(tc.tile_pool(name="rc", bufs=2))
        ob_pool = ctx.enter_context(tc.tile_pool(name="ob", bufs=4))
        proj_ps = ctx.enter_context(tc.tile_pool(name="proj_ps", bufs=2, space="PSUM"))
        s_ps = ctx.enter_context(tc.tile_pool(name="s_ps", bufs=2, space="PSUM"))
        pv_ps = ctx.enter_context(tc.tile_pool(name="pv_ps", bufs=2, space="PSUM"))
        out_ps = ctx.enter_context(tc.tile_pool(name="out_ps", bufs=2, space="PSUM"))

        # ---- constants / weights to SBUF
        wk_sb = const.tile([128, DC * 128], bf16, name="wk_sb")
        wq_sb = const.tile([128, DC * 128], bf16, name="wq_sb")
        wv_sb = const.tile([128, DC * 128], bf16, name="wv_sb")
        for dc in range(DC):
            nc.sync.dma_start(wk_sb[:, dc * 128:(dc + 1) * 128],
                              wk_d.ap()[dc * 128:(dc + 1) * 128, :])
            nc.sync.dma_start(wq_sb[:, dc * 128:(dc + 1) * 128],
                              wq_d.ap()[dc * 128:(dc + 1) * 128, :])
            nc.sync.dma_start(wv_sb[:, dc * 128:(dc + 1) * 128],
                              wv_d.ap()[dc * 128:(dc + 1) * 128, :])
        wo_sb = const.tile([128, D], bf16, name="wo_sb")
        nc.sync.dma_start(wo_sb[:], wo_d.ap())
        bk_sb = const.tile([128, 1], f32, name="bk_sb")
        nc.sync.dma_start(bk_sb[:], bk_d.ap())
        bq_sb = const.tile([128, 1], f32, name="bq_sb")
        nc.sync.dma_start(bq_sb[:], bq_d.ap())
        bvt_sb = const.tile([128, 128], f32, name="bvt_sb")
        nc.sync.dma_start(bvt_sb[:], bvt_d.ap())
        m01_sb = const.tile([128, 128], bf16, name="m01_sb")
        nc.sync.dma_start(m01_sb[:], m01_d.ap())

        for b in range(B):
            # ---- load x[b].T chunks
            xt_sb = []
            for dc in range(DC):
                t = xt_pool.tile([128, N], bf16, name=f"xt_b{b}_c{dc}", tag="xt")
                nc.sync.dma_start(t[:], xt_d.ap()[b, dc * 128:(dc + 1) * 128, :])
                xt_sb.append(t)

            # ---- k / q projections -> [128(2 heads x dh), N] bf16
            k2_sb = kq_pool.tile([128, N], bf16, name=f"k2_b{b}", tag="kq")
            q2_sb = kq_pool.tile([128, N], bf16, name=f"q2_b{b}", tag="kq")
            for w_sb, bias_sb, dst in ((wk_sb, bk_sb, k2_sb), (wq_sb, bq_sb, q2_sb)):
                for nj in range(NJ):
                    ps = proj_ps.tile([128, 512], f32, name="kq_ps", tag="proj")
                    for dc in range(DC):
                        nc.tensor.matmul(
                            ps[:], w_sb[:, dc * 128:(dc + 1) * 128],
                            xt_sb[dc][:, nj * 512:(nj + 1) * 512],
                            start=(dc == 0), stop=(dc == DC - 1))
                    nc.vector.tensor_scalar_add(
                        dst[:, nj * 512:(nj + 1) * 512], ps[:], bias_sb[:])

            # ---- v projection -> [n, 192-blocks: v_h0 | ones | v_h1] bf16
            v_sb = v_pool.tile([128, NB * 192], bf16, name=f"v_b{b}", tag="v")
            nc.vector.memset(v_sb[:], 1.0)
            for nb in range(NB):
                ps = proj_ps.tile([128, 128], f32, name="v_ps", tag="proj")
                for dc in range(DC):
                    nc.tensor.matmul(
                        ps[:], xt_sb[dc][:, nb * 128:(nb + 1) * 128],
                        wv_sb[:, dc * 128:(dc + 1) * 128],
                        start=(dc == 0), stop=(dc == DC - 1))
                # strided copy: psum cols [0:64]->v cols [0:64], [64:128]->[128:192]
                dst = v_sb[:, nb * 192:(nb + 1) * 192].rearrange(
                    "p (g c) -> p g c", g=3)[:, 0::2, :]
                nc.vector.tensor_tensor(
                    dst, ps[:].rearrange("p (g c) -> p g c", g=2),
                    bvt_sb[:].rearrange("p (g c) -> p g c", g=2),
                    mybir.AluOpType.add)

            # ---- attention (heads packed on PE row groups)
            sa_sb = sa_pool.tile([128, N], bf16, name=f"sa_b{b}", tag="sa")
            for j in range(NJ):
                pv = [pv_ps.tile([128, 512], f32, name=f"pv{h}", tag="pv")
                      for h in range(HL)]
                nch = 4 * (j + 1)
                for ci in range(nch):
                    t = ci - 4 * j
                    lo = 128 * t if t >= 0 else 0
                    for h in range(HL):
                        hp = 64 * h
                        sp = s_ps.tile([128, 512], f32, name="s", tag="s")
                        nc.tensor.matmul(
                            sp[:, lo:512],
                            q2_sb[hp:hp + 64, ci * 128:(ci + 1) * 128],
                            k2_sb[hp:hp + 64, j * 512 + lo:(j + 1) * 512],
                            start=True, stop=True)
                        pt = pt_pool.tile([128, 512], bf16, name="pt", tag="pt")
                        nc.scalar.activation(
                            pt[:, lo:512], sp[:, lo:512],
                            mybir.ActivationFunctionType.Exp, scale=0.125)
                        if t >= 0:
                            nc.vector.tensor_tensor(
                                pt[:, lo:lo + 128], pt[:, lo:lo + 128],
                                m01_sb[:], mybir.AluOpType.mult)
                        nc.tensor.matmul(
                            pv[h][:, lo:512],
                            v_sb[:, ci * 192 + 64 * h:ci * 192 + 64 * h + 128],
                            pt[:, lo:512],
                            start=(ci == 0), stop=(ci == nch - 1))
                for h in range(HL):
                    # h0: rows 0:64 = sa, 64:128 = denom ; h1: swapped
                    sa_rows = pv[h][64 * h:64 * h + 64, :]
                    den_rows = pv[h][64 - 64 * h:128 - 64 * h, :]
                    rc = rc_pool.tile([64, 512], f32, name="rc", tag="rc")
                    nc.vector.reciprocal(rc[:], den_rows)
                    nc.vector.tensor_tensor(
                        sa_sb[64 * h:64 * h + 64, j * 512:(j + 1) * 512],
                        sa_rows, rc[:], mybir.AluOpType.mult)

            # ---- output projection (partial over local heads)
            for nb in range(NB):
                for half in range(2):
                    op = out_ps.tile([128, 512], f32, name="op", tag="op")
                    nc.tensor.matmul(
                        op[:], sa_sb[:, nb * 128:(nb + 1) * 128],
                        wo_sb[:, half * 512:(half + 1) * 512],
                        start=True, stop=True)
                    ob = ob_pool.tile([128, 512], f32, name="ob", tag="ob")
                    if half == 0:
                        nc.scalar.copy(ob[:], op[:])
                    else:
                        nc.vector.tensor_copy(ob[:], op[:])
                    nc.sync.dma_start(
                        out_d.ap()[b, nb * 128:(nb + 1) * 128,
                                   half * 512:(half + 1) * 512], ob[:])

    nc.compile()
    return nc


def _get_nc():
    if "nc" not in _CACHE:
        _CACHE["nc"] = _build()
    return _CACHE["nc"]


def _prep_inputs(x, Wkqv, bkqv, Wo, bo):
    """Host-side shard prep: one input map per core."""
    xt = np.ascontiguousarray(x.transpose(0, 2, 1)).astype(BF16)
    tri = np.triu(np.ones((128, 128), np.float32)).astype(BF16)  # m' <= n''
    in_maps = []
    for c in range(NCORES):
        h0, h1 = HL * c, HL * c + 1
        wk2 = np.concatenate([Wkqv[h0, :, 0:64], Wkqv[h1, :, 0:64]], axis=1)
        wq2 = np.concatenate([Wkqv[h0, :, 64:128], Wkqv[h1, :, 64:128]], axis=1)
        wv2 = np.concatenate([Wkqv[h0, :, 128:192], Wkqv[h1, :, 128:192]], axis=1)
        bk2 = np.concatenate([bkqv[h0, 0:64], bkqv[h1, 0:64]])[:, None]
        bq2 = np.concatenate([bkqv[h0, 64:128], bkqv[h1, 64:128]])[:, None]
        bv2 = np.concatenate([bkqv[h0, 128:192], bkqv[h1, 128:192]])
        in_maps.append({
            "xt": xt,
            "wk2": wk2.astype(BF16),
            "wq2": wq2.astype(BF16),
            "wv2": wv2.astype(BF16),
            "wo2": Wo[128 * c:128 * (c + 1), :].astype(BF16),
            "bk2": np.ascontiguousarray(bk2, np.float32),
            "bq2": np.ascontiguousarray(bq2, np.float32),
            "bvt": np.ascontiguousarray(
                np.broadcast_to(bv2[None, :], (128, 128)), np.float32),
            "m01": tri,
        })
    return in_maps


def kernel(x, Wkqv, bkqv, Wo, bo):
    from concourse import bass_utils

    nc = _get_nc()
    in_maps = _prep_inputs(np.asarray(x), np.asarray(Wkqv), np.asarray(bkqv),
                           np.asarray(Wo), np.asarray(bo))
    res = bass_utils.run_bass_kernel_spmd(nc, in_maps, core_ids=list(range(NCORES)))
    acc = np.zeros((B, N, D), np.float32)
    for c in range(NCORES):
        acc += res.results[c]["out"]
    acc += np.asarray(bo)[None, None, :]
    return acc


# revision 13
# speedup vs baseline: 1.2889x; 1.2889x over previous
"""Causal self-attention (k/q swapped variant) on 8 Trainium2 NeuronCores.

Problem (hardcoded shapes): B=2, N=2048, D=1024, H=16, DH=64.
  kqv = einsum('bnd,hde->bhne', x, Wkqv) + bkqv   ; split -> k, q, v
  A[b,h,n,m] = k[b,h,n]·q[b,h,m] / sqrt(DH), causal mask m<=n, softmax over m
  sa = A @ v ; concat heads ; out = sa @ Wo + bo

Sharding: tensor-parallel over heads — each core owns 2 heads (both batch
entries), computes its partial output projection sa_local @ Wo[rows], and the
host sums the 8 partials (+bo).

Per-core device kernel layout (all matmul operands bf16, fp32 PSUM accum):
  - x is pre-transposed on host to xt[b] = x[b].T ([D, N]) so the contraction
    dim d lands on SBUF partitions.
  - k/q projections produce kT/qT in [dh, n] layout with both heads stacked on
    the partition dim (h0 rows 0:64, h1 rows 64:128) -> the two heads' K=64
    score matmuls occupy disjoint PE row groups and run concurrently.
  - scores are computed transposed, S^T[m, n] = q[m]·k[n], so softmax's
    reduction dim m sits on partitions; the denominator comes for free from
    the PV matmul by augmenting v with 64 columns of ones (psum rows 64:128
    hold the replicated row-sum). exp(S^T) only computed on the causal
    region; the 128x128 diagonal triangle is zeroed with a 0/1 mask multiply.
  - v is projected directly into [n, dh] layout (stationary = xt chunk).
  - output projection: stationary = saT column block, moving = Wo rows for the
    local heads -> natural-layout partial out [n, 1024] per block.
"""

import numpy as np
import ml_dtypes

B = 2
N = 2048
D = 1024
H = 16
DH = 64
NCORES = 8
HL = H // NCORES          # heads per core = 2
DC = D // 128             # contraction chunks = 8
NB = N // 128             # 128-row blocks = 16
NJ = N // 512             # 512-col blocks = 4

BF16 = ml_dtypes.bfloat16

_CACHE = {}


def _build():
    import concourse.bass as bass
    import concourse.mybir as mybir
    import concourse.tile as tile
    from concourse import bacc
    from contextlib import ExitStack

    f32 = mybir.dt.float32
    bf16 = mybir.dt.bfloat16

    nc = bacc.Bacc("TRN2", target_bir_lowering=False, debug=False,
                   enable_asserts=False, num_devices=NCORES)

    xt_d = nc.dram_tensor("xt", [B, D, N], bf16, kind="ExternalInput")
    wk_d = nc.dram_tensor("wk2", [D, 128], bf16, kind="ExternalInput")
    wq_d = nc.dram_tensor("wq2", [D, 128], bf16, kind="ExternalInput")
    wv_d = nc.dram_tensor("wv2", [D, 128], bf16, kind="ExternalInput")
    wo_d = nc.dram_tensor("wo2", [128, D], bf16, kind="ExternalInput")
    bk_d = nc.dram_tensor("bk2", [128, 1], f32, kind="ExternalInput")
    bq_d = nc.dram_tensor("bq2", [128, 1], f32, kind="ExternalInput")
    bvt_d = nc.dram_tensor("bvt", [128, 128], f32, kind="ExternalInput")
    m01_d = nc.dram_tensor("m01", [128, 128], bf16, kind="ExternalInput")
    out_d = nc.dram_tensor("out", [B, N, D], f32, kind="ExternalOutput")

    with tile.TileContext(nc) as tc, ExitStack() as ctx:
        const = ctx.enter_context(tc.tile_pool(name="const", bufs=1))
        xt_pool = ctx.enter_context(tc.tile_pool(name="xt", bufs=2 * DC))
        kq_pool = ctx.enter_context(tc.tile_pool(name="kq", bufs=4))
        v_pool = ctx.enter_context(tc.tile_pool(name="v", bufs=2))
        sa_pool = ctx.enter_context(tc.tile_pool(name="sa", bufs=2))
        pt_pool = ctx.enter_context(tc.tile_pool(name="pt", bufs=4))
        rc_pool = ctx.enter_context(tc.tile_pool(name="rc", bufs=2))
        ob_pool = ctx.enter_context(tc.tile_pool(name="ob", bufs=4))
        proj_ps = ctx.enter_context(tc.tile_pool(name="proj_ps", bufs=2, space="PSUM"))
        s_ps = ctx.enter_context(tc.tile_pool(name="s_ps", bufs=2, space="PSUM"))
        pv_ps = ctx.enter_context(tc.tile_pool(name="pv_ps", bufs=2, space="PSUM"))
        out_ps = ctx.enter_context(tc.tile_pool(name="out_ps", bufs=2, space="PSUM"))

        # ---- DMA issue order: kq weights -> xt batch 0 -> remaining consts
        # -> xt batch 1, so PE's first matmul group starts as early as
        # possible (single strided DMA per weight: DRAM [(dc p), m] ->
        # SBUF [p, (dc m)])
        xt_sb = {}

        def load_xt(b):
            for dc in range(DC):
                t = xt_pool.tile([128, N], bf16, name=f"xt_b{b}_c{dc}", tag="xt")
                nc.sync.dma_start(t[:], xt_d.ap()[b, dc * 128:(dc + 1) * 128, :])
                xt_sb[b, dc] = t

        wk_sb = const.tile([128, DC * 128], bf16, name="wk_sb")
        wq_sb = const.tile([128, DC * 128], bf16, name="wq_sb")
        wv_sb = const.tile([128, DC * 128], bf16, name="wv_sb")
        for w_sb, w_d in ((wk_sb, wk_d), (wq_sb, wq_d)):
            nc.sync.dma_start(
                w_sb[:].rearrange("p (dc m) -> p dc m", dc=DC),
                w_d.ap().rearrange("(dc p) m -> p dc m", p=128))
        load_xt(0)
        nc.sync.dma_start(
            wv_sb[:].rearrange("p (dc m) -> p dc m", dc=DC),
            wv_d.ap().rearrange("(dc p) m -> p dc m", p=128))
        wo_sb = const.tile([128, D], bf16, name="wo_sb")
        nc.sync.dma_start(wo_sb[:], wo_d.ap())
        bk_sb = const.tile([128, 1], f32, name="bk_sb")
        nc.sync.dma_start(bk_sb[:], bk_d.ap())
        bq_sb = const.tile([128, 1], f32, name="bq_sb")
        nc.sync.dma_start(bq_sb[:], bq_d.ap())
        bvt_sb = const.tile([128, 128], f32, name="bvt_sb")
        nc.sync.dma_start(bvt_sb[:], bvt_d.ap())
        m01_sb = const.tile([128, 128], bf16, name="m01_sb")
        nc.sync.dma_start(m01_sb[:], m01_d.ap())
        load_xt(1)

        for b in range(B):
            # ---- k / q projections -> [128(2 heads x dh), N] bf16
            k2_sb = kq_pool.tile([128, N], bf16, name=f"k2_b{b}", tag="kq")
            q2_sb = kq_pool.tile([128, N], bf16, name=f"q2_b{b}", tag="kq")
            for w_sb, bias_sb, dst in ((wk_sb, bk_sb, k2_sb), (wq_sb, bq_sb, q2_sb)):
                for nj in range(NJ):
                    ps = proj_ps.tile([128, 512], f32, name="kq_ps", tag="proj")
                    for dc in range(DC):
                        nc.tensor.matmul(
                            ps[:], w_sb[:, dc * 128:(dc + 1) * 128],
                            xt_sb[b, dc][:, nj * 512:(nj + 1) * 512],
                            start=(dc == 0), stop=(dc == DC - 1))
                    nc.vector.tensor_scalar_add(
                        dst[:, nj * 512:(nj + 1) * 512], ps[:], bias_sb[:])

            # ---- v projection -> [n, 192-blocks: v_h0 | ones | v_h1] bf16
            v_sb = v_pool.tile([128, NB * 192], bf16, name=f"v_b{b}", tag="v")
            nc.vector.memset(
                v_sb[:].rearrange("p (nb g) -> p nb g", g=192)[:, :, 64:128], 1.0)
            for nb in range(NB):
                ps = proj_ps.tile([128, 128], f32, name="v_ps", tag="proj")
                for dc in range(DC):
                    nc.tensor.matmul(
                        ps[:], xt_sb[b, dc][:, nb * 128:(nb + 1) * 128],
                        wv_sb[:, dc * 128:(dc + 1) * 128],
                        start=(dc == 0), stop=(dc == DC - 1))
                # strided copy: psum cols [0:64]->v cols [0:64], [64:128]->[128:192]
                dst = v_sb[:, nb * 192:(nb + 1) * 192].rearrange(
                    "p (g c) -> p g c", g=3)[:, 0::2, :]
                nc.vector.tensor_tensor(
                    dst, ps[:].rearrange("p (g c) -> p g c", g=2),
                    bvt_sb[:].rearrange("p (g c) -> p g c", g=2),
                    mybir.AluOpType.add)

            # ---- attention (heads packed on PE row groups)
            sa_sb = sa_pool.tile([128, N], bf16, name=f"sa_b{b}", tag="sa")
            for j in range(NJ):
                pv = [pv_ps.tile([128, 512], f32, name=f"pv{h}", tag="pv")
                      for h in range(HL)]
                nch = 4 * (j + 1)
                for ci in range(nch):
                    t = ci - 4 * j
                    lo = 128 * t if t >= 0 else 0
                    for h in range(HL):
                        hp = 64 * h
                        sp = s_ps.tile([128, 512], f32, name="s", tag="s")
                        nc.tensor.matmul(
                            sp[:, lo:512],
                            q2_sb[hp:hp + 64, ci * 128:(ci + 1) * 128],
                            k2_sb[hp:hp + 64, j * 512 + lo:(j + 1) * 512],
                            start=True, stop=True)
                        pt = pt_pool.tile([128, 512], bf16, name="pt", tag="pt")
                        nc.scalar.activation(
                            pt[:, lo:512], sp[:, lo:512],
                            mybir.ActivationFunctionType.Exp, scale=0.125)
                        if t >= 0:
                            nc.gpsimd.tensor_tensor(
                                pt[:, lo:lo + 128], pt[:, lo:lo + 128],
                                m01_sb[:], mybir.AluOpType.mult)
                        nc.tensor.matmul(
                            pv[h][:, lo:512],
                            v_sb[:, ci * 192 + 64 * h:ci * 192 + 64 * h + 128],
                            pt[:, lo:512],
                            start=(ci == 0), stop=(ci == nch - 1))
                for h in range(HL):
                    # h0: rows 0:64 = sa, 64:128 = denom ; h1: swapped
                    sa_rows = pv[h][64 * h:64 * h + 64, :]
                    den_rows = pv[h][64 - 64 * h:128 - 64 * h, :]
                    # denominators are sums of exp() terms in [~2e-3, ~3e3]:
                    # safely inside approx_fast's domain; 18-bit accuracy is
                    # far below the bf16 noise floor of the P*V numerator.
                    # (approx_fast misreads PSUM operands on HW - bounce the
                    # denominator row block through SBUF first.)
                    den_sb = rc_pool.tile([64, 512], f32, name="den", tag="den")
                    nc.vector.tensor_copy(den_sb[:], den_rows)
                    rc = rc_pool.tile([64, 512], f32, name="rc", tag="rc")
                    nc.vector.reciprocal_approx_fast(rc[:], den_sb[:])
                    nc.vector.tensor_tensor(
                        sa_sb[64 * h:64 * h + 64, j * 512:(j + 1) * 512],
                        sa_rows, rc[:], mybir.AluOpType.mult)

            # ---- output projection (partial over local heads)
            for nb in range(NB):
                for half in range(2):
                    op = out_ps.tile([128, 512], f32, name="op", tag="op")
                    nc.tensor.matmul(
                        op[:], sa_sb[:, nb * 128:(nb + 1) * 128],
                        wo_sb[:, half * 512:(half + 1) * 512],
                        start=True, stop=True)
                    ob = ob_pool.tile([128, 512], f32, name="ob", tag="ob")
                    if half == 0:
                        nc.scalar.copy(ob[:], op[:])
                    else:
                        nc.vector.tensor_copy(ob[:], op[:])
                    nc.sync.dma_start(
                        out_d.ap()[b, nb * 128:(nb + 1) * 128,
                                   half * 512:(half + 1) * 512], ob[:])

    nc.compile()
    return nc


def _get_nc():
    if "nc" not in _CACHE:
        _CACHE["nc"] = _build()
    return _CACHE["nc"]


def _prep_inputs(x, Wkqv, bkqv, Wo, bo):
    """Host-side shard prep: one input map per core."""
    xt = np.ascontiguousarray(x.transpose(0, 2, 1)).astype(BF16)
    tri = np.triu(np.ones((128, 128), np.float32)).astype(BF16)  # m' <= n''
    in_maps = []
    for c in range(NCORES):
        h0, h1 = HL * c, HL * c + 1
        wk2 = np.concatenate([Wkqv[h0, :, 0:64], Wkqv[h1, :, 0:64]], axis=1)
        wq2 = np.concatenate([Wkqv[h0, :, 64:128], Wkqv[h1, :, 64:128]], axis=1)
        wv2 = np.concatenate([Wkqv[h0, :, 128:192], Wkqv[h1, :, 128:192]], axis=1)
        bk2 = np.concatenate([bkqv[h0, 0:64], bkqv[h1, 0:64]])[:, None]
        bq2 = np.concatenate([bkqv[h0, 64:128], bkqv[h1, 64:128]])[:, None]
        bv2 = np.concatenate([bkqv[h0, 128:192], bkqv[h1, 128:192]])
        in_maps.append({
            "xt": xt,
            "wk2": wk2.astype(BF16),
            "wq2": wq2.astype(BF16),
            "wv2": wv2.astype(BF16),
            "wo2": Wo[128 * c:128 * (c + 1), :].astype(BF16),
            "bk2": np.ascontiguousarray(bk2, np.float32),
            "bq2": np.ascontiguousarray(bq2, np.float32),
            "bvt": np.ascontiguousarray(
                np.broadcast_to(bv2[None, :], (128, 128)), np.float32),
            "m01": tri,
        })
    return in_maps


def kernel(x, Wkqv, bkqv, Wo, bo):
    from concourse import bass_utils

    nc = _get_nc()
    in_maps = _prep_inputs(np.asarray(x), np.asarray(Wkqv), np.asarray(bkqv),
                           np.asarray(Wo), np.asarray(bo))
    res = bass_utils.run_bass_kernel_spmd(nc, in_maps, core_ids=list(range(NCORES)))
    acc = np.zeros((B, N, D), np.float32)
    for c in range(NCORES):
        acc += res.results[c]["out"]
    acc += np.asarray(bo)[None, None, :]
    return acc
